# revision 58
# baseline (speedup 1.0000x reference)
"""Trainium2 Bass kernel for nn_BasicBlock (spiking CNN block).

Sharding: data-parallel over batch B across 8 NeuronCores (4 batch x 4
timesteps = 16 images per core); BN batch stats via tiny AllReduce.

Per core, 16 matmul pass-equivalents vs the naive 27 (hi/lo fp16 split):
- conv1 main (W1hi . xhi, fp16): taps row-paired via planes laid out as
  [xhi ; xhi shifted one row] so K=128 contracts two taps -> 6 passes.
- conv1 corr (W1hi . xlo + W1lo . xhi): fp8e4 DoubleRow matmuls (0.5
  cyc/row) over planes [e4m3(x) ; e4m3(xlo*2^11)], two taps per
  instruction (k-tile strides must be EVEN or the ifmap fetcher dies),
  M=128 with half-zero weight columns so each image's correction lands
  on its own partition half of ONE corr PSUM bank. Streams flat at the
  padded pitch (58); pad columns hold garbage the evacuation skips.
  2.5 pass-equivalents.
- conv2 on fp8 spike planes [s1 ; s1 shifted] (s1 exact in fp8): main =
  f16-weight x fp8-moving tap-paired passes (6), W2lo corr = fp8 DR
  (1.5). One fp8 plane serves both; halves the plane DMA bytes.
- Evacuation: Act copy (main psum) -> strip, DVE stt strip += corr*2^-11
  (accum_out -> BN sums), Act square (accum_out -> BN sumsq).
- Stats: 8-op chain (th = std*K1 + mean, gm = beta*rsc - mean with
  host-folded K1); swap-halves read straight from the collective's DRAM
  output in parallel with the main readback.
- PLIF scans keep negated state Pneg=-P so spikes and state updates are
  single fused scalar_tensor_tensor ops; reset mask via Act (s-1).
  gpsimd ops must never write in-place (device hangs).
- Residual+LIF2: Pool computes wneg = Pneg - y2 off the critical path,
  Act scales x by rsc2 in-place, DVE does compare/state; spikes leave
  as fp16 pair-DMAs (both images of a pair in one 128-partition DMA)
  and are cast on host.
"""
import sys
sys.path.insert(0, '/opt/trn_rl_repo')

import numpy as np

T, B, C, H, W = 4, 32, 64, 56, 56
NCORES = 8
BL = B // NCORES            # 4 local batch samples
NIMG = T * BL               # 16 images per core
HP = W + 2                  # 58
PPP = HP * HP + 4           # padded plane + flat-stream overrun guard (3368)
PIX = H * W                 # 3136
NCH = 7                     # conv chunks per image (8 rows each)
CHW = 8 * W                 # 448 compact chunk
CFL = 8 * HP                # 464 flat chunk span
NPAIR = 8                   # image pairs per core
EPS = 1e-5
NG = float((T * B) * PIX)   # 401408
QL = 14 * W                 # LIF quarter-strip length (784)
NQ = 4
CSC = 2048.0                # 2^11 fp8 correction scale

# (di, dj) slice per fp16 main set: di=0 -> taps (0,dj)+(1,dj) paired via
# the shifted upper half; di=2 -> tap (2,dj) solo (upper weights zero).
MAIN_SETS = [(0, 0), (0, 1), (0, 2), (2, 0), (2, 1), (2, 2)]
# conv1 fp8-DR sets: (di, dj, dk) -> k-tile1 at (di,dj), k-tile2 at +dk
# covering tap pairs ((0,j),(1,j))x3, ((2,0),(2,2)) [stride 2; odd k-tile
# strides crash the ifmap fetcher], ((2,1), zero)
DR1_SETS = [(0, 0, HP), (0, 1, HP), (0, 2, HP), (2, 0, 2), (2, 1, 2)]
# conv2 fp8-DR sets: k-tiles 2 rows apart cover taps (0,j),(1,j),(2,j),x0
DR2_SETS = [(0, 0, 2 * HP), (0, 1, 2 * HP), (0, 2, 2 * HP)]

_prog_cache = {}
DBG = False
NO_CC = False
PHASES = 3
TRACE = False
LAST_RES = None
LAST_NAMES = None
LAST_EXEC_NS = None


def _build(alpha1, alpha2):
    import concourse.mybir as mybir
    import concourse.tile as tile
    from concourse.ap import AP
    from concourse import bacc

    F32 = mybir.dt.float32
    F16 = mybir.dt.float16
    F8 = mybir.dt.float8e4
    AO = mybir.AluOpType
    AF = mybir.ActivationFunctionType
    AX = mybir.AxisListType
    DRM = mybir.MatmulPerfMode.DoubleRow

    c1 = 1.0 - alpha1
    c2 = 1.0 - alpha2

    def sub_ap(base, extra_off, dims):
        b = base.copy()
        return AP(b.tensor, b.offset + extra_off,
                  [list(b.ap[0])] + [list(d) for d in dims])

    nc = bacc.Bacc(None, target_bir_lowering=False)
    names = {}

    with tile.TileContext(nc) as tc:
        with tc.tile_pool(name="dram", bufs=1, space="DRAM") as dram:
            pa = dram.tile([NIMG, 128, PPP], F16, kind="ExternalInput")
            pb = dram.tile([NIMG, 128, PPP], F8, kind="ExternalInput")
            xin = dram.tile([NIMG, 64, PIX], F32, kind="ExternalInput")
            w1m = dram.tile([128, 6, 64], F16, kind="ExternalInput")
            w1c = dram.tile([128, 5, 2, 192], F8, kind="ExternalInput")
            w2m = dram.tile([128, 6, 64], F16, kind="ExternalInput")
            w2c = dram.tile([128, 3, 2, 192], F8, kind="ExternalInput")
            cpar = dram.tile([128, 8], F32, kind="ExternalInput")
            outp = dram.tile([NIMG, 64, PIX], F16, kind="ExternalOutput")
            names.update(pa=pa.name, pb=pb.name, xin=xin.name, w1m=w1m.name,
                         w1c=w1c.name, w2m=w2m.name, w2c=w2c.name,
                         cpar=cpar.name, outp=outp.name)
            if DBG:
                y1d = dram.tile([NPAIR, 128, PIX], F32, kind="ExternalOutput")
                y2d = dram.tile([NPAIR, 128, PIX], F32, kind="ExternalOutput")
                s1d = dram.tile([NPAIR, 128, PIX], F32, kind="ExternalOutput")
                vecd = dram.tile([128, 8], F32, kind="ExternalOutput")
                names.update(y1d=y1d.name, y2d=y2d.name, s1d=s1d.name,
                             vecd=vecd.name)

            with tc.tile_pool(name="dramw", bufs=1, space="DRAM") as dramw, \
                 tc.tile_pool(name="wsb", bufs=1) as wsb, \
                 tc.tile_pool(name="ys", bufs=8) as yspool, \
                 tc.tile_pool(name="plane", bufs=4) as plpool, \
                 tc.tile_pool(name="hfp", bufs=2) as hf, \
                 tc.tile_pool(name="tiny", bufs=17) as tiny, \
                 tc.tile_pool(name="ps", bufs=7, space="PSUM") as ps:

                # ---- static parameter loads
                w1ms = wsb.tile([128, 6, 64], F16, tag="w1m")
                nc.sync.dma_start(w1ms[:], w1m[:])
                w1cs = wsb.tile([128, 5, 2, 192], F8, tag="w1c")
                nc.sync.dma_start(w1cs[:], w1c[:])
                w2ms = wsb.tile([128, 6, 64], F16, tag="w2m")
                nc.sync.dma_start(w2ms[:], w2m[:])
                w2cs = wsb.tile([128, 3, 2, 192], F8, tag="w2c")
                nc.sync.dma_start(w2cs[:], w2c[:])
                cpars = wsb.tile([128, 8], F32, tag="cpar")
                nc.sync.dma_start(cpars[:], cpar[:])
                negone = wsb.tile([128, 1], F32, tag="negone")
                nc.vector.memset(negone[:], -1.0)
                zq = wsb.tile([128, QL], F32, tag="zq")
                nc.vector.memset(zq[:], 0.0)
                epst = wsb.tile([128, 1], F32, tag="epst")
                nc.vector.memset(epst[:], EPS)
                sums1 = wsb.tile([128, 56], F32, tag="sums1")
                sums1q = wsb.tile([128, 56], F32, tag="sums1q")
                sums2 = wsb.tile([128, 56], F32, tag="sums2")
                sums2q = wsb.tile([128, 56], F32, tag="sums2q")
                if PHASES < 2:
                    nc.vector.memset(sums2[:], 0.0)
                    nc.vector.memset(sums2q[:], 0.0)

                def conv_img_pair(plA, plB, plA8, plB8, wm, wc, dr_sets,
                                  dst_strip, sums_t, sumsq_t, pcol):
                    """One image pair: per chunk, 6 fp16 tap-paired matmuls
                    per image into a main psum + fp8 DoubleRow correction
                    instructions (both images) into one flat corr psum, then
                    the 3-op evacuation."""
                    plAr = plA[:, 0:HP * HP].rearrange("p (r w) -> p r w", w=HP)
                    plBr = plB[:, 0:HP * HP].rearrange("p (r w) -> p r w", w=HP)
                    ndr = len(dr_sets)
                    for cth in range(NCH):
                        r0 = 8 * cth
                        pm = ps.tile([128, CHW], F32, tag="psm", bufs=5,
                                     name=f"psm{cth & 1}")
                        pc = ps.tile([128, CFL], F32, tag="psc", bufs=3,
                                     name=f"psc{cth & 1}")
                        for si, (di, dj) in enumerate(MAIN_SETS):
                            for j, plr in enumerate((plAr, plBr)):
                                rhs = plr[:, r0 + di:r0 + di + 8, dj:dj + W]
                                out = pm[64 * j:64 * (j + 1), :] \
                                    .rearrange("p (r w) -> p r w", r=8)
                                nc.tensor.matmul(
                                    out, wm[:, si, :], rhs,
                                    start=(si == 0), stop=(si == 5),
                                    tile_position=(0, 64 * j),
                                    skip_group_check=True)
                        idx = 0
                        for j, pl8 in enumerate((plA8, plB8)):
                            co = 64 * (1 - j)
                            for i, (di, dj, dk) in enumerate(dr_sets):
                                base = (r0 + di) * HP + dj
                                rhs = sub_ap(pl8[:], base, [(dk, 2), (1, CFL)])
                                nc.tensor.matmul(
                                    pc[:], wc[:, i, :, co:co + 128], rhs,
                                    start=(idx == 0), stop=(idx == 2 * ndr - 1),
                                    perf_mode=DRM, tile_position=(0, 0),
                                    skip_group_check=True)
                                idx += 1
                        # evacuation: copy main, add scaled corr, square
                        sl = dst_strip[:, CHW * cth:CHW * (cth + 1)]
                        slv = sl.rearrange("p (r w) -> p r w", w=W)
                        nc.scalar.activation(sl, pm[:], AF.Copy)
                        pcv = sub_ap(pc[:], 0, [(HP, 8), (1, W)])
                        nc.vector.scalar_tensor_tensor(
                            slv, pcv, 1.0 / CSC, slv, AO.mult, AO.add,
                            accum_out=sums_t[:, pcol * 7 + cth:pcol * 7 + cth + 1])
                        jk = hf.tile([128, CHW], F16, tag="jk", bufs=1)
                        nc.scalar.activation(
                            jk[:], sl, AF.Square,
                            accum_out=sumsq_t[:, pcol * 7 + cth:pcol * 7 + cth + 1])


                # ================= phase A: conv1 =================
                y1s = []
                for p in range(NPAIR):
                    tt_, bp = p // 2, p % 2
                    iA = tt_ * 4 + bp * 2
                    pls, pl8s_ = [], []
                    for j in range(2):
                        i = iA + j
                        ta = plpool.tile([128, PPP], F16, tag="plf16")
                        t8 = plpool.tile([128, PPP], F8, tag="plf8")
                        if p == 0:
                            hh = 30 * HP
                            nc.sync.dma_start(ta[:, 0:hh], pa[i, :, 0:hh])
                            nc.sync.dma_start(ta[:, hh:PPP], pa[i, :, hh:PPP])
                            nc.scalar.dma_start(t8[:, 0:hh], pb[i, :, 0:hh])
                            nc.scalar.dma_start(t8[:, hh:PPP], pb[i, :, hh:PPP])
                        else:
                            nc.sync.dma_start(ta[:], pa[i])
                            nc.scalar.dma_start(t8[:], pb[i])
                        pls.append(ta)
                        pl8s_.append(t8)
                    strip = yspool.tile([128, PIX], F32, tag="ys")
                    y1s.append(strip)
                    conv_img_pair(pls[0], pls[1], pl8s_[0], pl8s_[1],
                                  w1ms, w1cs, DR1_SETS, strip, sums1, sums1q, p)
                    if DBG:
                        nc.sync.dma_start(y1d[p], strip[:])

                # ---- stats1 allreduce
                cc1i = dramw.tile([128, 2], F32)
                cc1o = dramw.tile([128, 2], F32, addr_space="Shared")
                acc1 = tiny.tile([128, 2], F32, tag="acc")
                nc.vector.tensor_reduce(acc1[:, 0:1], sums1[:], AX.X, AO.add)
                nc.vector.tensor_reduce(acc1[:, 1:2], sums1q[:], AX.X, AO.add)
                nc.sync.dma_start(cc1i[:], acc1[:])
                if NO_CC:
                    nc.sync.dma_start(cc1o[:], cc1i[:])
                else:
                    nc.gpsimd.collective_compute(
                        "AllReduce", AO.add, ins=[cc1i[:]], outs=[cc1o[:]],
                        replica_groups=[list(range(NCORES))])

                def stats_block(cco, beta, rgam, k1):
                    g = tiny.tile([128, 2], F32, tag="acc")
                    nc.sync.dma_start(g[:], cco[:])
                    gr = tiny.tile([128, 2], F32, tag="acc")
                    nc.scalar.dma_start(gr[0:64, :], cco[64:128, :])
                    nc.scalar.dma_start(gr[64:128, :], cco[0:64, :])
                    tot = tiny.tile([128, 2], F32, tag="acc")
                    nc.vector.tensor_tensor(tot[:], g[:], gr[:], AO.add)
                    mean = tiny.tile([128, 1], F32, tag="t1")
                    nc.vector.tensor_scalar(mean[:], tot[:, 0:1], 1.0 / NG,
                                            None, AO.mult)
                    msq = tiny.tile([128, 1], F32, tag="t1")
                    nc.vector.tensor_scalar(msq[:], tot[:, 1:2], 1.0 / NG,
                                            None, AO.mult)
                    m2 = tiny.tile([128, 1], F32, tag="t1")
                    nc.vector.scalar_tensor_tensor(m2[:], mean[:], 1.0, mean[:],
                                                   AO.bypass, AO.mult)
                    var = tiny.tile([128, 1], F32, tag="t1")
                    nc.vector.tensor_tensor(var[:], msq[:], m2[:], AO.subtract)
                    std = tiny.tile([128, 1], F32, tag="t1")
                    nc.scalar.activation(std[:], var[:], AF.Sqrt, bias=epst[:])
                    rscv = tiny.tile([128, 1], F32, tag="t1")
                    nc.vector.tensor_tensor(rscv[:], std[:], rgam, AO.mult)
                    gamv = tiny.tile([128, 1], F32, tag="t1")
                    # gm = beta*rsc - mean
                    nc.vector.scalar_tensor_tensor(gamv[:], rscv[:], beta,
                                                   mean[:], AO.mult,
                                                   AO.subtract)
                    th = tiny.tile([128, 1], F32, tag="t1")
                    # th = std*K1 + mean, K1 = rga*(1 - alpha*beta)
                    nc.vector.scalar_tensor_tensor(th[:], std[:], k1,
                                                   mean[:], AO.mult, AO.add)
                    return th, gamv, rscv

                th1, gm1, _rsc1 = stats_block(
                    cc1o, cpars[:, 0:1], cpars[:, 2:3], cpars[:, 4:5])
                # wv1 bias: c1*gm1 so wv = c1*(q + gm1); Pneg = wv*(s-1) = -P
                gm1c1 = tiny.tile([128, 1], F32, tag="t1")
                nc.vector.tensor_scalar(gm1c1[:], gm1[:], c1, None, AO.mult)
                if DBG:
                    nc.sync.dma_start(vecd[:, 0:1], th1[:])
                    nc.sync.dma_start(vecd[:, 1:2], gm1[:])
                    nc.sync.dma_start(vecd[:, 4:5], acc1[:, 0:1])
                    nc.sync.dma_start(vecd[:, 5:6], acc1[:, 1:2])

                # ============ phase B + C: LIF1 + conv2 ============
                y2s = [None] * NPAIR
                for bp in range(2 if PHASES >= 2 else 0):
                    Pneg = [None] * NQ
                    for t in range(1, 5):
                        p = (t - 1) * 2 + bp
                        s1tq = []
                        for hq in range(NQ):
                            off = QL * hq
                            ysl = y1s[p][:, off:off + QL]
                            s1t8 = hf.tile([128, QL], F8, tag="s1t8",
                                           bufs=2)
                            if t == 1:
                                nc.vector.tensor_scalar(s1t8[:], ysl, th1[:],
                                                        None, AO.is_ge)
                            else:
                                # s = (y1 - th) >= Pneg  <=>  y1 + P >= th
                                nc.vector.scalar_tensor_tensor(
                                    s1t8[:], ysl, th1[:], Pneg[hq][:],
                                    AO.subtract, AO.is_ge)
                            s1tq.append(s1t8)
                            if DBG:
                                nc.gpsimd.dma_start(
                                    s1d[p, :, off:off + QL], s1t8[:])
                            if t < 4:
                                mng = hf.tile([128, QL], F16, tag="mng", bufs=3)
                                nc.scalar.activation(mng[:], s1t8[:],
                                                     AF.Identity,
                                                     bias=negone[:])
                                # v = q + gm1 = (y1 + gm1) - Pneg
                                v = hf.tile([128, QL], F32, tag="wvn", bufs=2)
                                if t == 1:
                                    nc.vector.tensor_scalar(
                                        v[:], ysl, gm1[:], None, AO.add)
                                else:
                                    nc.vector.scalar_tensor_tensor(
                                        v[:], ysl, gm1[:], Pneg[hq][:],
                                        AO.add, AO.subtract)
                                Pn = hf.tile([128, QL], F32, tag="pp", bufs=8)
                                # (c1*v) * (s-1) = -c1*v*(1-s) = -P'
                                nc.vector.scalar_tensor_tensor(
                                    Pn[:], v[:], c1, mng[:], AO.mult, AO.mult)
                                Pneg[hq] = Pn
                        # fp8 spike planes [s1 ; s1 shifted one row] serve
                        # both the f16-weight main passes and the fp8 DR corr
                        s8_pair = []
                        for j in range(2):
                            s8 = plpool.tile([128, PPP], F8, tag="plf8")
                            s8r = s8[:, 0:HP * HP].rearrange(
                                "p (r w) -> p r w", w=HP)
                            for hq in range(NQ):
                                s1t8 = s1tq[hq]
                                src8 = s1t8[64 * j:64 * (j + 1), :] \
                                    .rearrange("p (r w) -> p r w", w=W)
                                rl = 1 + 14 * hq
                                ru = 14 * hq
                                nc.sync.dma_start(
                                    s8r[0:64, rl:rl + 14, 1:1 + W], src8)
                                nc.scalar.dma_start(
                                    s8r[64:128, ru:ru + 14, 1:1 + W], src8)
                            s8_pair.append(s8)
                        strip2 = yspool.tile([128, PIX], F32, tag="ys")
                        y2s[p] = strip2
                        conv_img_pair(s8_pair[0], s8_pair[1], s8_pair[0],
                                      s8_pair[1], w2ms, w2cs, DR2_SETS,
                                      strip2, sums2, sums2q, p)
                        if DBG:
                            nc.sync.dma_start(y2d[p], strip2[:])

                # ---- stats2 allreduce
                cc2i = dramw.tile([128, 2], F32)
                cc2o = dramw.tile([128, 2], F32, addr_space="Shared")
                acc2 = tiny.tile([128, 2], F32, tag="acc")
                nc.vector.tensor_reduce(acc2[:, 0:1], sums2[:], AX.X, AO.add)
                nc.vector.tensor_reduce(acc2[:, 1:2], sums2q[:], AX.X, AO.add)
                nc.sync.dma_start(cc2i[:], acc2[:])
                if NO_CC:
                    nc.sync.dma_start(cc2o[:], cc2i[:])
                else:
                    nc.gpsimd.collective_compute(
                        "AllReduce", AO.add, ins=[cc2i[:]], outs=[cc2o[:]],
                        replica_groups=[list(range(NCORES))])
                th2, gm2, rsc2 = stats_block(
                    cc2o, cpars[:, 1:2], cpars[:, 3:4], cpars[:, 5:6])
                if DBG:
                    nc.sync.dma_start(vecd[:, 2:3], th2[:])
                    nc.sync.dma_start(vecd[:, 3:4], gm2[:])

                # ============ phase D: residual + LIF2 ============
                # t-outer emission: 8 independent (bp,hq) chains per step
                Pneg2 = {}
                for t in range(1 if PHASES >= 3 else 5, 5):
                    for bp in range(2):
                        p = (t - 1) * 2 + bp
                        iA = (t - 1) * 4 + bp * 2
                        for hq in range(NQ):
                            off = QL * hq
                            # wneg = Pneg - y2 on Pool, off the x path
                            wneg = hf.tile([128, QL], F32, tag="wvn", bufs=2)
                            if t == 1:
                                nc.gpsimd.tensor_tensor(
                                    wneg[:], zq[:], y2s[p][:, off:off + QL],
                                    AO.subtract)
                            else:
                                nc.gpsimd.tensor_tensor(
                                    wneg[:], Pneg2[bp, hq][:],
                                    y2s[p][:, off:off + QL], AO.subtract)
                            xs = hf.tile([128, QL], F32, tag="xs", bufs=5)
                            nc.sync.dma_start(
                                xs[:],
                                xin[iA:iA + 2, :, off:off + QL]
                                .rearrange("a p q -> (a p) q"))
                            # xsc = x*rsc2 in-place (Act)
                            nc.scalar.activation(xs[:], xs[:], AF.Copy,
                                                 scale=rsc2[:])
                            ot = hf.tile([128, QL], F16, tag="ot", bufs=3)
                            # s: (xsc - th) >= wneg  <=>  xsc + y2 + P >= th
                            nc.vector.scalar_tensor_tensor(
                                ot[:], xs[:], th2[:], wneg[:],
                                AO.subtract, AO.is_ge)
                            nc.scalar.dma_start(
                                outp[iA:iA + 2, :, off:off + QL]
                                .rearrange("a p q -> (a p) q"), ot[:])
                            if t < 4:
                                mng = hf.tile([128, QL], F16, tag="mng",
                                              bufs=3)
                                nc.scalar.activation(mng[:], ot[:],
                                                     AF.Identity,
                                                     bias=negone[:])
                                # u = q + gm2 = (xsc + gm2) - wneg, in-place
                                nc.vector.scalar_tensor_tensor(
                                    xs[:], xs[:], gm2[:], wneg[:],
                                    AO.add, AO.subtract)
                                Pn = hf.tile([128, QL], F32, tag="pp", bufs=8)
                                # (c2*u) * (s-1) = -c2*u*(1-s) = -P'
                                nc.vector.scalar_tensor_tensor(
                                    Pn[:], xs[:], c2, mng[:], AO.mult, AO.mult)
                                Pneg2[bp, hq] = Pn

    nc.compile()
    return nc, names


def _sigmoid(x):
    return 1.0 / (1.0 + np.exp(-float(x)))


def prepare(x, conv1_w, bn1_gamma, bn1_beta, lif1_w, conv2_w, bn2_gamma,
            bn2_beta, lif2_w):
    import ml_dtypes
    E4 = ml_dtypes.float8_e4m3

    x = np.ascontiguousarray(np.asarray(x, np.float32))
    conv1_w = np.asarray(conv1_w, np.float32)
    conv2_w = np.asarray(conv2_w, np.float32)

    a1 = _sigmoid(np.asarray(lif1_w).reshape(-1)[0])
    a2 = _sigmoid(np.asarray(lif2_w).reshape(-1)[0])

    key = (round(a1, 12), round(a2, 12))
    if key not in _prog_cache:
        _prog_cache[key] = _build(a1, a2)
    nc, names = _prog_cache[key]

    xh = x.astype(np.float16)
    xl = x - xh.astype(np.float32)

    # fp16 main planes: [xhi padded ; xhi shifted up one row]
    pa_np = np.zeros((T, B, 128, HP, HP), np.float16)
    pa_np[:, :, 0:64, 1:57, 1:57] = xh
    pa_np[:, :, 64:128, 0:56, 1:57] = xh
    # fp8 corr planes: [e4m3(x) ; e4m3(xl*2^11)], same (unshifted) layout
    pb_np = np.zeros((T, B, 128, HP, HP), E4)
    pb_np[:, :, 0:64, 1:57, 1:57] = x.astype(E4)
    pb_np[:, :, 64:128, 1:57, 1:57] = (xl * CSC).astype(E4)

    def pad_flat(arr):
        # [T,B,128,HP,HP] -> [T,B,128,PPP] with zero tail
        t_, b_, p_, _, _ = arr.shape
        out = np.zeros((t_, b_, p_, PPP), arr.dtype)
        out[..., :HP * HP] = arr.reshape(t_, b_, p_, HP * HP)
        return out

    pa_np = pad_flat(pa_np)
    pb_np = pad_flat(pb_np)

    w1h = conv1_w.astype(np.float16)
    w1l = conv1_w - w1h.astype(np.float32)
    w2h = conv2_w.astype(np.float16)
    w2l = conv2_w - w2h.astype(np.float32)

    def main_stack(wh):
        out = np.zeros((128, 6, 64), np.float16)
        for s, (di, dj) in enumerate(MAIN_SETS):
            if di == 0:
                out[0:64, s] = wh[:, :, 0, dj].T
                out[64:128, s] = wh[:, :, 1, dj].T
            else:
                out[0:64, s] = wh[:, :, 2, dj].T
        return out

    w1m_np = main_stack(w1h)

    w2m_np = main_stack(w2h)
    w2l_s = (w2l * CSC).astype(E4)
    w2c_np = np.zeros((128, 3, 2, 192), E4)
    for dj in range(3):
        w2c_np[0:64, dj, 0, 64:128] = w2l_s[:, :, 0, dj].astype(
            np.float32).T.astype(E4)
        w2c_np[64:128, dj, 0, 64:128] = w2l_s[:, :, 1, dj].astype(
            np.float32).T.astype(E4)
        w2c_np[0:64, dj, 1, 64:128] = w2l_s[:, :, 2, dj].astype(
            np.float32).T.astype(E4)

    # conv1 DR weights: k-tile block rows 0:64 multiply e4m3(x) -> W1l_s,
    # rows 64:128 multiply e4m3(xl*2^11) -> W1h8; active cols at 64:128.
    w1l_s = (w1l * CSC).astype(E4)
    w1h8 = conv1_w.astype(E4)
    w1c_np = np.zeros((128, 5, 2, 192), E4)
    dr1_taps = [((0, 0), (1, 0)), ((0, 1), (1, 1)), ((0, 2), (1, 2)),
                ((2, 0), (2, 2)), ((2, 1), None)]
    for i, (ta, tb) in enumerate(dr1_taps):
        for kk, tap in enumerate((ta, tb)):
            if tap is None:
                continue
            di, dj = tap
            w1c_np[0:64, i, kk, 64:128] = w1l_s[:, :, di, dj].astype(
                np.float32).T.astype(E4)
            w1c_np[64:128, i, kk, 64:128] = w1h8[:, :, di, dj].astype(
                np.float32).T.astype(E4)

    def dup(v):
        v = np.asarray(v, np.float32).reshape(64)
        return np.concatenate([v, v])

    cpar_np = np.zeros((128, 8), np.float32)
    cpar_np[:, 0] = dup(bn1_beta)
    cpar_np[:, 1] = dup(bn2_beta)
    cpar_np[:, 2] = 1.0 / dup(bn1_gamma)
    cpar_np[:, 3] = 1.0 / dup(bn2_gamma)
    rga1 = 1.0 / (a1 * dup(bn1_gamma))
    rga2 = 1.0 / (a2 * dup(bn2_gamma))
    cpar_np[:, 4] = rga1 * (1.0 - a1 * dup(bn1_beta))
    cpar_np[:, 5] = rga2 * (1.0 - a2 * dup(bn2_beta))

    in_maps = []
    for k in range(NCORES):
        sl = slice(4 * k, 4 * k + 4)
        pa_k = np.ascontiguousarray(pa_np[:, sl].reshape(NIMG, 128, PPP))
        pb_k = np.ascontiguousarray(pb_np[:, sl].reshape(NIMG, 128, PPP))
        xin_k = np.ascontiguousarray(x[:, sl].reshape(NIMG, 64, PIX))
        in_maps.append({
            names['pa']: pa_k,
            names['pb']: pb_k.view(np.uint8),
            names['xin']: xin_k,
            names['w1m']: w1m_np,
            names['w1c']: w1c_np.view(np.uint8),
            names['w2m']: w2m_np,
            names['w2c']: w2c_np.view(np.uint8),
            names['cpar']: cpar_np,
        })

    return nc, names, in_maps


def kernel(**inputs):
    from concourse.bass_utils import run_bass_kernel_spmd
    nc, names, in_maps = prepare(**inputs)
    res = run_bass_kernel_spmd(nc, in_maps, core_ids=list(range(NCORES)))
    global LAST_RES, LAST_NAMES
    LAST_RES, LAST_NAMES = res, names
    out = np.empty((T, B, C, H, W), np.float32)
    for k in range(NCORES):
        o = np.asarray(res.results[k][names['outp']], np.float32)
        out[:, 4 * k:4 * k + 4] = o.reshape(T, BL, C, H, W)
    return out


if __name__ == "__main__":
    rng = np.random.default_rng(0)
    xs = rng.standard_normal((T, B, C, H, W)).astype(np.float32)
    w1 = (rng.standard_normal((64, 64, 3, 3)) * 0.05).astype(np.float32)
    w2 = (rng.standard_normal((64, 64, 3, 3)) * 0.05).astype(np.float32)
    o = kernel(x=xs, conv1_w=w1, bn1_gamma=np.ones(64, np.float32),
               bn1_beta=np.zeros(64, np.float32),
               lif1_w=np.zeros(1, np.float32), conv2_w=w2,
               bn2_gamma=np.ones(64, np.float32),
               bn2_beta=np.zeros(64, np.float32),
               lif2_w=np.zeros(1, np.float32))
    print("ran:", o.shape, float(o.mean()))
